# revision 20
# baseline (speedup 1.0000x reference)
"""Bahdanau additive attention on 8 Trainium2 NeuronCores.

Reference computation (per batch b):
    q = query[b] @ W1                      # (TQ, U)
    k = value[b] @ W2                      # (TK, U)
    scores[t,s] = sum_u scale[u] * tanh(q[t,u] + k[s,u])
    attn = softmax(scores + mask_bias, axis=s)
    context = attn @ value[b]              # (TQ, NH)

Sharding: pure data-parallel over batch (B=8 == n_cores). Each core gets
its own batch slice plus replicated W1/W2/scale; no collectives.

Per-core dataflow (partition dim = u for the cube stages):
    PE:   qT[u,t] = W1^T q^T,  kT[u,s] = W2^T v^T   (contraction over h)
    DVE:  S[u, (t,s)] = qT[u,t] (bcast over s) + kT[u,s] (bcast over t)
    ACT:  T = tanh(S)  (bf16)
    PE:   scoresT[s, t] (PSUM) += tanh_slice(u,s)^T @ scale_half(u,1),
          accumulated over the two u-halves, one matmul per (t, half)
    ACT:  expT = exp(scoresT + maskb[s])   (mask folded into the bias)
    PE:   ctx_unnorm(t,h) = expT^T @ v ; sums(t,1) = expT^T @ ones
          exp(t,s) = transpose(expT)
    DVE:  rinv = 1/sums ; attn = exp * rinv ; ctx = ctx_unnorm * rinv

Sync-wait discipline: walrus allows very few sem waits per compute
instruction, so inputs are packed into 4 DMAs (4 HW queue semaphores),
PSUM->SBUF copies run on ACT so the DVE adds wait on one semaphore, all
S/T tiles stay resident (no slot-reuse waits), and dummy ops pre-consume
watermarks ahead of the fp32 matmuls/transposes.
"""

import numpy as np

import concourse.bass as bass
import concourse.mybir as mybir
import concourse.tile as tile
from concourse import bacc
from concourse.bass_utils import run_bass_kernel_spmd

F32 = mybir.dt.float32
BF16 = mybir.dt.bfloat16

P = 128          # partitions
B = 8            # batch == n_cores
TQ = 128         # query positions
TK = 128         # key positions
NH = 512         # model dim
U = 256          # attention units
HC = NH // P     # h chunks (4)
UH = U // P      # u halves (2)
TCH = 16         # t-chunk per DVE-add / ACT-tanh op (free dim 16*128=2048)
NCH = TQ // TCH  # 8 chunks
NEG_INF = -1e9

PK = TQ + U              # packed projection width (384)
# vaux packed columns: [ value(512) | maskb(1) | scale(2) | identity(128) ]
VA_V, VA_MB, VA_SC, VA_ID = 0, NH, NH + 1, NH + 3
VA_W = NH + 3 + P        # 643


def _bcast_free(sub, n, inner):
    """Insert a broadcast (step-0) free dim into an AP.

    inner=True appends [0, n] as the innermost free dim; inner=False puts
    it as the outermost free dim (right after the partition dim).
    """
    ap = [list(d) for d in sub.ap]
    if inner:
        new = ap + [[0, n]]
    else:
        new = [ap[0], [0, n]] + ap[1:]
    return bass.AP(tensor=sub.tensor, offset=sub.offset, ap=new)


def build_program():
    nc = bacc.Bacc("TRN2", target_bir_lowering=False)

    # p1 = [query_b^T | W1], p2 = [value_b^T | W2] packed host-side so each
    # projection chain depends on a single DMA queue (fp32 self-loading
    # matmuls have one sync-wait slot). vaux packs value/mask/scale/identity.
    p1 = nc.dram_tensor("p1", [NH, PK], F32, kind="ExternalInput").ap()
    p2 = nc.dram_tensor("p2", [NH, PK], F32, kind="ExternalInput").ap()
    vaux = nc.dram_tensor("vaux", [TK, VA_W], F32, kind="ExternalInput").ap()
    out_t = nc.dram_tensor("out", [TQ, NH + TK], F32, kind="ExternalOutput").ap()

    with tile.TileContext(nc) as tc:
        with (
            tc.tile_pool(name="consts", bufs=1) as consts,
            tc.tile_pool(name="spool", bufs=NCH * UH) as spool,
            tc.tile_pool(name="tpool", bufs=NCH * UH) as tpool,
            tc.tile_pool(name="soft", bufs=1) as soft,
            tc.tile_pool(name="pproj", bufs=1, space="PSUM") as pproj,
            tc.tile_pool(name="psc", bufs=1, space="PSUM") as psc,
            tc.tile_pool(name="ptail", bufs=1, space="PSUM") as ptail,
        ):
            # ---- input loads: exactly 4 DMAs -> 4 HW queue semaphores ----
            p1_sb = consts.tile([P, HC, PK], F32, tag="p1")
            p2_sb = consts.tile([P, HC, PK], F32, tag="p2")
            va_sb = consts.tile([P, VA_W], F32, tag="vaux")

            # Chunked loads (one DMA per h-chunk) so the projection matmuls
            # start as soon as their first chunk lands.
            p1_r = p1.rearrange("(c p) f -> p c f", p=P)
            p2_r = p2.rearrange("(c p) f -> p c f", p=P)
            for hc in range(HC):
                nc.sync.dma_start(out=p1_sb[:, hc, :], in_=p1_r[:, hc, :])
                nc.sync.dma_start(out=p2_sb[:, hc, :], in_=p2_r[:, hc, :])
            nc.sync.dma_start(out=va_sb, in_=vaux)

            qt_sb = p1_sb[:, :, 0:TQ]
            w1_sb = p1_sb[:, :, TQ:PK]
            vt_sb = p2_sb[:, :, 0:TK]
            w2_sb = p2_sb[:, :, TK:PK]
            v_sb = va_sb[:, VA_V : VA_V + NH]
            maskb_sb = va_sb[:, VA_MB : VA_MB + 1]
            scale_f32 = va_sb[:, VA_SC : VA_SC + UH]
            ident = va_sb[:, VA_ID : VA_ID + P]

            scale_bf = consts.tile([P, UH], BF16, tag="scalebf")
            nc.vector.tensor_copy(scale_bf, scale_f32)
            ones_sb = consts.tile([P, 1], F32, tag="ones")
            nc.vector.memset(ones_sb, 1.0)

            # Warm the ACT table set (tanh/exp share "exp_and_others") while
            # the input DMAs are in flight, and pre-consume the vaux queue
            # watermark on ACT (for the exp bias = maskb later).
            warm = soft.tile([P, 1], F32, tag="warm")
            nc.vector.memset(warm, 0.0)
            nc.scalar.activation(warm, warm, mybir.ActivationFunctionType.Tanh)
            warm2 = soft.tile([P, 1], F32, tag="warm2")
            nc.scalar.copy(warm2, maskb_sb)

            # Dummy 1-element matmul pre-consuming the vaux queue watermark
            # on PE (v / ident feed later fp32 matmuls with 1 wait slot).
            scratch_ps = ptail.tile([1, 1], F32, tag="scratch")
            nc.tensor.matmul(
                scratch_ps, lhsT=v_sb[:, 0:1], rhs=v_sb[:, 0:1],
                start=True, stop=True,
            )

            # ---- projections: qT[u,t], kT[u,s] (PE, contraction over h) ----
            psq = pproj.tile([P, UH, TQ], F32, tag="psq")
            psk = pproj.tile([P, UH, TK], F32, tag="psk")
            for uh in range(UH):
                for hc in range(HC):
                    nc.tensor.matmul(
                        psq[:, uh, :],
                        lhsT=w1_sb[:, hc, uh * P : (uh + 1) * P],
                        rhs=qt_sb[:, hc, :],
                        start=(hc == 0),
                        stop=(hc == HC - 1),
                    )
            for uh in range(UH):
                for hc in range(HC):
                    nc.tensor.matmul(
                        psk[:, uh, :],
                        lhsT=w2_sb[:, hc, uh * P : (uh + 1) * P],
                        rhs=vt_sb[:, hc, :],
                        start=(hc == 0),
                        stop=(hc == HC - 1),
                    )
            # PSUM -> SBUF copies on DVE (ACT is the cube bottleneck; Bacc's
            # generate_event_semaphores legalizes any multi-waits).
            qT_sb = consts.tile([P, UH, TQ], F32, tag="qT")
            kT_sb = consts.tile([P, UH, TK], F32, tag="kT")
            nc.vector.tensor_copy(qT_sb, psq)
            nc.vector.tensor_copy(kT_sb, psk)

            # ---- main cube: S = q (+) k, tanh, scale-reduce over u ----
            scT_ps = psc.tile([P, TQ], F32, tag="scT")

            for ci in range(NCH):
                t0 = ci * TCH
                tanh_t = []
                for uh in range(UH):
                    # bf16 S keeps all 16 tiles resident (no slot reuse ->
                    # no extra sync waits); tanh input rounding ~2e-3.
                    s_t = spool.tile([P, TCH, TK], BF16, tag="S")
                    kb = _bcast_free(kT_sb[:, uh, :], TCH, inner=False)
                    qb = _bcast_free(qT_sb[:, uh, t0 : t0 + TCH], TK, inner=True)
                    # Offload ~1/3 of the broadcast-adds to the otherwise
                    # idle GpSimd engine; DVE keeps the rest.
                    op = ci * UH + uh
                    eng = nc.gpsimd if op % 3 == 2 else nc.vector
                    eng.tensor_add(s_t, kb, qb)
                    t_t = tpool.tile([P, TCH, TK], BF16, tag="T")
                    nc.scalar.activation(t_t, s_t, mybir.ActivationFunctionType.Tanh)
                    tanh_t.append(t_t)
                for tloc in range(TCH):
                    t = t0 + tloc
                    for uh in range(UH):
                        nc.tensor.matmul(
                            scT_ps[:, t : t + 1],
                            lhsT=tanh_t[uh][:, tloc, :],
                            rhs=scale_bf[:, uh : uh + 1],
                            start=(uh == 0),
                            stop=(uh == UH - 1),
                        )

            # ---- masked softmax over s + context ----
            expT_sb = soft.tile([P, TQ], F32, tag="expT")
            nc.scalar.activation(
                expT_sb, scT_ps, mybir.ActivationFunctionType.Exp, bias=maskb_sb
            )
            ctx_ps = ptail.tile([P, NH], F32, tag="ctx")
            sums_ps = ptail.tile([P, 1], F32, tag="sums")
            exp_ps = ptail.tile([P, TK], F32, tag="exp")
            nc.tensor.matmul(ctx_ps, lhsT=expT_sb, rhs=v_sb, start=True, stop=True)
            nc.tensor.matmul(sums_ps, lhsT=expT_sb, rhs=ones_sb, start=True, stop=True)
            nc.tensor.transpose(exp_ps, expT_sb, ident)
            rinv = soft.tile([P, 1], F32, tag="rinv")
            nc.vector.reciprocal(rinv, sums_ps)
            # DVE shim: consume the transpose's PE watermark so the muls
            # below carry at most one sync wait each.
            shim = soft.tile([P, 1], F32, tag="shim")
            nc.vector.tensor_copy(shim, exp_ps[:, 0:1])
            outbuf = soft.tile([P, NH + TK], F32, tag="outbuf")
            nc.vector.tensor_scalar_mul(outbuf[:, NH : NH + TK], exp_ps, rinv)
            nc.vector.tensor_scalar_mul(outbuf[:, 0:NH], ctx_ps, rinv)
            nc.sync.dma_start(out=out_t, in_=outbuf)

    nc.compile()
    return nc


_NC_CACHE = None


def _get_program():
    global _NC_CACHE
    if _NC_CACHE is None:
        _NC_CACHE = build_program()
    return _NC_CACHE


def make_in_maps(query, value, mask, W1, W2, scale):
    maskb = np.where(mask, 0.0, NEG_INF).astype(np.float32)
    w1 = np.asarray(W1, dtype=np.float32)
    w2 = np.asarray(W2, dtype=np.float32)
    sc2 = np.asarray(scale, dtype=np.float32).reshape(UH, P).T  # (128, 2)
    eye = np.eye(P, dtype=np.float32)
    in_maps = []
    for b in range(B):
        vaux = np.concatenate(
            [
                np.asarray(value[b], dtype=np.float32),
                maskb[b][:, None],
                sc2,
                eye,
            ],
            axis=1,
        )
        in_maps.append(
            {
                "p1": np.ascontiguousarray(
                    np.concatenate([query[b].T, w1], axis=1), dtype=np.float32
                ),
                "p2": np.ascontiguousarray(
                    np.concatenate([value[b].T, w2], axis=1), dtype=np.float32
                ),
                "vaux": np.ascontiguousarray(vaux),
            }
        )
    return in_maps


def kernel(query, value, mask, W1, W2, scale, **run_kwargs):
    query = np.asarray(query)
    value = np.asarray(value)
    mask = np.asarray(mask)
    nc = _get_program()
    in_maps = make_in_maps(query, value, mask, W1, W2, scale)
    res = run_bass_kernel_spmd(nc, in_maps, list(range(B)), **run_kwargs)
    context = np.stack([res.results[b]["out"][:, 0:NH] for b in range(B)])
    attn = np.stack([res.results[b]["out"][:, NH : NH + TK] for b in range(B)])
    kernel.last_results = res
    return context, attn


# revision 25
# speedup vs baseline: 1.1174x; 1.1174x over previous
"""Bahdanau additive attention on 8 Trainium2 NeuronCores.

Reference computation (per batch b):
    q = query[b] @ W1                      # (TQ, U)
    k = value[b] @ W2                      # (TK, U)
    scores[t,s] = sum_u scale[u] * tanh(q[t,u] + k[s,u])
    attn = softmax(scores + mask_bias, axis=s)
    context = attn @ value[b]              # (TQ, NH)

Sharding: pure data-parallel over batch (B=8 == n_cores). Each core gets
its own batch slice plus replicated W1/W2/scale; no collectives.

Per-core dataflow (partition dim = u for the cube stages):
    PE:   qT[u,t] = W1^T q^T,  kT[u,s] = W2^T v^T   (contraction over h)
    DVE:  S[u, (t,s)] = qT[u,t] (bcast over s) + kT[u,s] (bcast over t)
    ACT:  T = tanh(S)  (bf16)
    PE:   scoresT[s, t] (PSUM) += tanh_slice(u,s)^T @ scale_half(u,1),
          accumulated over the two u-halves, one matmul per (t, half)
    per t-half (so the first half's tail hides under the second half's
    cube work):
      ACT:  expT = exp(scoresT + maskb[s])   (mask folded into the bias)
      PE:   ctx_unnorm(t,h) = expT^T @ v ; sums(t,1) = expT^T @ ones
            exp(t,s) = transpose(expT)
      DVE:  rinv = 1/sums ; attn = exp * rinv ; ctx = ctx_unnorm * rinv

Host packs inputs partition-major so each DMA descriptor covers a multi-KB
contiguous run (the DMA engines are descriptor-rate-bound otherwise).
"""

import numpy as np

import concourse.bass as bass
import concourse.mybir as mybir
import concourse.tile as tile
from concourse import bacc
from concourse.bass_utils import run_bass_kernel_spmd

F32 = mybir.dt.float32
BF16 = mybir.dt.bfloat16

P = 128          # partitions
B = 8            # batch == n_cores
TQ = 128         # query positions
TK = 128         # key positions
NH = 512         # model dim
U = 256          # attention units
HC = NH // P     # h chunks (4)
UH = U // P      # u halves (2)
TCH = 16         # t-chunk per DVE-add / ACT-tanh op (free dim 16*128=2048)
NCH = TQ // TCH  # 8 chunks
TH = TQ // 2     # tail processed per t-half (64)
NEG_INF = -1e9

PK = TQ + U              # packed projection width (384)
# vaux packed columns: [ value(512) | maskb(1) | scale(2) | identity(128) ]
VA_V, VA_MB, VA_SC, VA_ID = 0, NH, NH + 1, NH + 3
VA_W = NH + 3 + P        # 643


def _bcast_free(sub, n, inner):
    """Insert a broadcast (step-0) free dim into an AP.

    inner=True appends [0, n] as the innermost free dim; inner=False puts
    it as the outermost free dim (right after the partition dim).
    """
    ap = [list(d) for d in sub.ap]
    if inner:
        new = ap + [[0, n]]
    else:
        new = [ap[0], [0, n]] + ap[1:]
    return bass.AP(tensor=sub.tensor, offset=sub.offset, ap=new)


def build_program():
    nc = bacc.Bacc("TRN2", target_bir_lowering=False)

    # p1 = [query_b^T | W1], p2 = [value_b^T | W2], both stored
    # partition-major on the host: row p holds the four h-chunks
    # (h = c*128 + p) back to back -> 1.5KB-6KB contiguous DMA runs.
    p1 = nc.dram_tensor("p1", [P, HC * PK], F32, kind="ExternalInput").ap()
    p2 = nc.dram_tensor("p2", [P, HC * PK], F32, kind="ExternalInput").ap()
    vaux = nc.dram_tensor("vaux", [TK, VA_W], F32, kind="ExternalInput").ap()
    out_t = nc.dram_tensor("out", [TQ, NH + TK], F32, kind="ExternalOutput").ap()

    with tile.TileContext(nc) as tc:
        with (
            tc.tile_pool(name="consts", bufs=1) as consts,
            tc.tile_pool(name="spool", bufs=NCH * UH) as spool,
            tc.tile_pool(name="tpool", bufs=NCH * UH) as tpool,
            tc.tile_pool(name="soft", bufs=1) as soft,
            tc.tile_pool(name="pproj", bufs=1, space="PSUM") as pproj,
            tc.tile_pool(name="psc", bufs=1, space="PSUM") as psc,
            tc.tile_pool(name="ptail", bufs=1, space="PSUM") as ptail,
        ):
            # ---- input loads ----
            p1_sb = consts.tile([P, HC, PK], F32, tag="p1")
            p2_sb = consts.tile([P, HC, PK], F32, tag="p2")
            va_sb = consts.tile([P, VA_W], F32, tag="vaux")

            # Two DMAs per projection tensor (3KB descriptor runs) so the
            # first half's projections can start while the rest streams in.
            half = HC // 2 * PK
            nc.sync.dma_start(out=p1_sb[:, 0 : HC // 2, :], in_=p1[:, 0:half])
            nc.sync.dma_start(out=p2_sb[:, 0 : HC // 2, :], in_=p2[:, 0:half])
            nc.sync.dma_start(out=p1_sb[:, HC // 2 : HC, :], in_=p1[:, half:])
            nc.sync.dma_start(out=p2_sb[:, HC // 2 : HC, :], in_=p2[:, half:])
            nc.sync.dma_start(out=va_sb, in_=vaux)

            qt_sb = p1_sb[:, :, 0:TQ]
            w1_sb = p1_sb[:, :, TQ:PK]
            vt_sb = p2_sb[:, :, 0:TK]
            w2_sb = p2_sb[:, :, TK:PK]
            v_sb = va_sb[:, VA_V : VA_V + NH]
            maskb_sb = va_sb[:, VA_MB : VA_MB + 1]
            scale_f32 = va_sb[:, VA_SC : VA_SC + UH]
            ident = va_sb[:, VA_ID : VA_ID + P]

            scale_bf = consts.tile([P, UH], BF16, tag="scalebf")
            nc.vector.tensor_copy(scale_bf, scale_f32)
            # Warm the ACT table set (tanh/exp share "exp_and_others") while
            # the input DMAs are in flight; also touch maskb on ACT early.
            warm = soft.tile([P, 1], F32, tag="warm")
            nc.vector.memset(warm, 0.0)
            nc.scalar.activation(warm, warm, mybir.ActivationFunctionType.Tanh)
            warm2 = soft.tile([P, 1], F32, tag="warm2")
            nc.scalar.copy(warm2, maskb_sb)

            # ---- projections: qT[u,t], kT[u,s] (PE, contraction over h) ----
            psq = pproj.tile([P, UH, TQ], F32, tag="psq")
            psk = pproj.tile([P, UH, TK], F32, tag="psk")
            for uh in range(UH):
                for hc in range(HC):
                    nc.tensor.matmul(
                        psq[:, uh, :],
                        lhsT=w1_sb[:, hc, uh * P : (uh + 1) * P],
                        rhs=qt_sb[:, hc, :],
                        start=(hc == 0),
                        stop=(hc == HC - 1),
                    )
            for uh in range(UH):
                for hc in range(HC):
                    nc.tensor.matmul(
                        psk[:, uh, :],
                        lhsT=w2_sb[:, hc, uh * P : (uh + 1) * P],
                        rhs=vt_sb[:, hc, :],
                        start=(hc == 0),
                        stop=(hc == HC - 1),
                    )
            # PSUM -> SBUF copies on ACT (DVE is the cube bottleneck).
            qT_sb = consts.tile([P, UH, TQ], F32, tag="qT")
            kT_sb = consts.tile([P, UH, TK], F32, tag="kT")
            nc.scalar.copy(qT_sb, psq)
            nc.scalar.copy(kT_sb, psk)

            # ---- main cube + per-half softmax/context tail ----
            # Separate PSUM tiles per t-half so the first half's exp read
            # doesn't serialize against the second half's matmul writes.
            scT = [
                psc.tile([P, TH], F32, tag="scT0", name="scT0"),
                psc.tile([P, TH], F32, tag="scT1", name="scT1"),
            ]
            expT_sb = soft.tile([P, TQ], F32, tag="expT")
            # Per-half tail tiles, all at partition base 0 (transpose matmul
            # outputs must start at PSUM partition 0); the output DMA remaps
            # rows to the right half of out_t.
            ctxp = [
                ptail.tile([TH, NH], F32, tag="ctx0", name="ctx0"),
                ptail.tile([TH, NH], F32, tag="ctx1", name="ctx1"),
            ]
            expp = [
                ptail.tile([TH, TK], F32, tag="exp0", name="exp0"),
                ptail.tile([TH, TK], F32, tag="exp1", name="exp1"),
            ]
            sums = [
                soft.tile([TH, 1], F32, tag="sums0", name="sums0"),
                soft.tile([TH, 1], F32, tag="sums1", name="sums1"),
            ]
            rinv = [
                soft.tile([TH, 1], F32, tag="rinv0", name="rinv0"),
                soft.tile([TH, 1], F32, tag="rinv1", name="rinv1"),
            ]
            outb = [
                soft.tile([TH, NH + TK], F32, tag="outb0", name="outb0"),
                soft.tile([TH, NH + TK], F32, tag="outb1", name="outb1"),
            ]

            for ci in range(NCH):
                t0 = ci * TCH
                th = t0 // TH          # which t-half this chunk belongs to
                tl = t0 % TH           # position within the half
                tanh_t = []
                for uh in range(UH):
                    # bf16 S keeps all 16 tiles resident (no slot reuse ->
                    # no extra sync waits); tanh input rounding ~2e-3.
                    s_t = spool.tile([P, TCH, TK], BF16, tag="S")
                    kb = _bcast_free(kT_sb[:, uh, :], TCH, inner=False)
                    qb = _bcast_free(qT_sb[:, uh, t0 : t0 + TCH], TK, inner=True)
                    nc.vector.tensor_add(s_t, kb, qb)
                    t_t = tpool.tile([P, TCH, TK], BF16, tag="T")
                    nc.scalar.activation(t_t, s_t, mybir.ActivationFunctionType.Tanh)
                    tanh_t.append(t_t)
                for tloc in range(TCH):
                    for uh in range(UH):
                        nc.tensor.matmul(
                            scT[th][:, tl + tloc : tl + tloc + 1],
                            lhsT=tanh_t[uh][:, tloc, :],
                            rhs=scale_bf[:, uh : uh + 1],
                            start=(uh == 0),
                            stop=(uh == UH - 1),
                        )

                if tl + TCH == TH:
                    # This t-half's scores are complete: run its masked
                    # softmax + context now so it overlaps the next half.
                    h0 = th * TH
                    esl = expT_sb[:, h0 : h0 + TH]
                    nc.scalar.activation(
                        esl, scT[th], mybir.ActivationFunctionType.Exp,
                        bias=maskb_sb,
                    )
                    nc.tensor.matmul(
                        ctxp[th], lhsT=esl, rhs=v_sb, start=True, stop=True
                    )
                    nc.tensor.transpose(expp[th], esl, ident)
                    nc.vector.reduce_sum(
                        sums[th], expp[th], axis=mybir.AxisListType.X
                    )
                    nc.vector.reciprocal(rinv[th], sums[th])
                    nc.vector.tensor_scalar_mul(
                        outb[th][:, NH : NH + TK], expp[th], rinv[th]
                    )
                    nc.vector.tensor_scalar_mul(
                        outb[th][:, 0:NH], ctxp[th], rinv[th]
                    )
                    nc.sync.dma_start(out=out_t[h0 : h0 + TH, :], in_=outb[th])

    nc.compile()
    return nc


_NC_CACHE = None


def _get_program():
    global _NC_CACHE
    if _NC_CACHE is None:
        _NC_CACHE = build_program()
    return _NC_CACHE


def make_in_maps(query, value, mask, W1, W2, scale):
    maskb = np.where(mask, 0.0, NEG_INF).astype(np.float32)
    w1 = np.asarray(W1, dtype=np.float32)
    w2 = np.asarray(W2, dtype=np.float32)
    sc2 = np.asarray(scale, dtype=np.float32).reshape(UH, P).T  # (128, 2)
    eye = np.eye(P, dtype=np.float32)
    in_maps = []
    for b in range(B):
        # (NH, PK) h-major -> (P, HC*PK) partition-major
        p1 = np.concatenate([query[b].T, w1], axis=1).reshape(HC, P, PK)
        p2 = np.concatenate([value[b].T, w2], axis=1).reshape(HC, P, PK)
        p1 = np.ascontiguousarray(p1.transpose(1, 0, 2).reshape(P, HC * PK))
        p2 = np.ascontiguousarray(p2.transpose(1, 0, 2).reshape(P, HC * PK))
        vaux = np.ascontiguousarray(
            np.concatenate(
                [np.asarray(value[b], dtype=np.float32), maskb[b][:, None], sc2, eye],
                axis=1,
            )
        )
        in_maps.append({"p1": p1.astype(np.float32), "p2": p2.astype(np.float32),
                        "vaux": vaux})
    return in_maps


def kernel(query, value, mask, W1, W2, scale, **run_kwargs):
    query = np.asarray(query)
    value = np.asarray(value)
    mask = np.asarray(mask)
    nc = _get_program()
    in_maps = make_in_maps(query, value, mask, W1, W2, scale)
    res = run_bass_kernel_spmd(nc, in_maps, list(range(B)), **run_kwargs)
    context = np.stack([res.results[b]["out"][:, 0:NH] for b in range(B)])
    attn = np.stack([res.results[b]["out"][:, NH : NH + TK] for b in range(B)])
    kernel.last_results = res
    return context, attn


# revision 27
# speedup vs baseline: 1.2236x; 1.0950x over previous
"""Bahdanau additive attention on 8 Trainium2 NeuronCores.

Reference computation (per batch b):
    q = query[b] @ W1                      # (TQ, U)
    k = value[b] @ W2                      # (TK, U)
    scores[t,s] = sum_u scale[u] * tanh(q[t,u] + k[s,u])
    attn = softmax(scores + mask_bias, axis=s)
    context = attn @ value[b]              # (TQ, NH)

Sharding: pure data-parallel over batch (B=8 == n_cores). Each core gets
its own batch slice plus replicated W1/W2/scale; no collectives.

Per-core dataflow (partition dim = u for the cube stages):
    PE:   qT[u,t] = W1^T q^T,  kT[u,s] = W2^T v^T   (contraction over h)
    DVE:  S[u, (t,s)] = qT[u,t] (bcast over s) + kT[u,s] (bcast over t)
    ACT:  T = tanh(S)  (bf16)
    PE:   scoresT[s, t] (PSUM) += tanh_slice(u,s)^T @ scale_half(u,1),
          accumulated over the two u-halves, one matmul per (t, half)
    per t-half (so the first half's tail hides under the second half's
    cube work):
      ACT:  expT = exp(scoresT + maskb[s])   (mask folded into the bias)
      PE:   ctx_unnorm(t,h) = expT^T @ v ; sums(t,1) = expT^T @ ones
            exp(t,s) = transpose(expT)
      DVE:  rinv = 1/sums ; attn = exp * rinv ; ctx = ctx_unnorm * rinv

Host packs inputs partition-major so each DMA descriptor covers a multi-KB
contiguous run (the DMA engines are descriptor-rate-bound otherwise).
"""

import numpy as np

import concourse.bass as bass
import concourse.mybir as mybir
import concourse.tile as tile
from concourse import bacc
from concourse.bass_utils import run_bass_kernel_spmd

F32 = mybir.dt.float32
BF16 = mybir.dt.bfloat16

P = 128          # partitions
B = 8            # batch == n_cores
TQ = 128         # query positions
TK = 128         # key positions
NH = 512         # model dim
U = 256          # attention units
HC = NH // P     # h chunks (4)
UH = U // P      # u halves (2)
TCH = 16         # t-chunk per DVE-add / ACT-tanh op (free dim 16*128=2048)
NCH = TQ // TCH  # 8 chunks
TH = TQ // 2     # tail processed per t-half (64)
NEG_INF = -1e9

PK = TQ + U              # packed projection width (384)
# vaux packed columns: [ value(512) | maskb(1) | scale(2) | identity(128) ]
VA_V, VA_MB, VA_SC, VA_ID = 0, NH, NH + 1, NH + 3
VA_W = NH + 3 + P        # 643


def _bcast_free(sub, n, inner):
    """Insert a broadcast (step-0) free dim into an AP.

    inner=True appends [0, n] as the innermost free dim; inner=False puts
    it as the outermost free dim (right after the partition dim).
    """
    ap = [list(d) for d in sub.ap]
    if inner:
        new = ap + [[0, n]]
    else:
        new = [ap[0], [0, n]] + ap[1:]
    return bass.AP(tensor=sub.tensor, offset=sub.offset, ap=new)


def build_program():
    nc = bacc.Bacc("TRN2", target_bir_lowering=False)

    # p1 = [query_b^T | W1], p2 = [value_b^T | W2], both stored
    # partition-major on the host: row p holds the four h-chunks
    # (h = c*128 + p) back to back -> 1.5KB-6KB contiguous DMA runs.
    p1 = nc.dram_tensor("p1", [P, HC * PK], BF16, kind="ExternalInput").ap()
    p2 = nc.dram_tensor("p2", [P, HC * PK], BF16, kind="ExternalInput").ap()
    vaux = nc.dram_tensor("vaux", [TK, VA_W], F32, kind="ExternalInput").ap()
    out_t = nc.dram_tensor("out", [TQ, NH + TK], F32, kind="ExternalOutput").ap()

    with tile.TileContext(nc) as tc:
        with (
            tc.tile_pool(name="consts", bufs=1) as consts,
            tc.tile_pool(name="spool", bufs=NCH * UH) as spool,
            tc.tile_pool(name="tpool", bufs=NCH * UH) as tpool,
            tc.tile_pool(name="soft", bufs=1) as soft,
            tc.tile_pool(name="pproj", bufs=1, space="PSUM") as pproj,
            tc.tile_pool(name="psc", bufs=1, space="PSUM") as psc,
            tc.tile_pool(name="ptail", bufs=1, space="PSUM") as ptail,
        ):
            # ---- input loads ----
            p1_sb = consts.tile([P, HC, PK], BF16, tag="p1")
            p2_sb = consts.tile([P, HC, PK], BF16, tag="p2")
            va_sb = consts.tile([P, VA_W], F32, tag="vaux")

            # Two DMAs per projection tensor (3KB descriptor runs) so the
            # first half's projections can start while the rest streams in.
            half = HC // 2 * PK
            nc.sync.dma_start(out=p1_sb[:, 0 : HC // 2, :], in_=p1[:, 0:half])
            nc.sync.dma_start(out=p2_sb[:, 0 : HC // 2, :], in_=p2[:, 0:half])
            nc.sync.dma_start(out=p1_sb[:, HC // 2 : HC, :], in_=p1[:, half:])
            nc.sync.dma_start(out=p2_sb[:, HC // 2 : HC, :], in_=p2[:, half:])
            nc.sync.dma_start(out=va_sb, in_=vaux)

            qt_sb = p1_sb[:, :, 0:TQ]
            w1_sb = p1_sb[:, :, TQ:PK]
            vt_sb = p2_sb[:, :, 0:TK]
            w2_sb = p2_sb[:, :, TK:PK]
            v_sb = va_sb[:, VA_V : VA_V + NH]
            maskb_sb = va_sb[:, VA_MB : VA_MB + 1]
            scale_f32 = va_sb[:, VA_SC : VA_SC + UH]
            ident = va_sb[:, VA_ID : VA_ID + P]

            scale_bf = consts.tile([P, UH], BF16, tag="scalebf")
            nc.vector.tensor_copy(scale_bf, scale_f32)
            # Warm the ACT table set (tanh/exp share "exp_and_others") while
            # the input DMAs are in flight; also touch maskb on ACT early.
            warm = soft.tile([P, 1], F32, tag="warm")
            nc.vector.memset(warm, 0.0)
            nc.scalar.activation(warm, warm, mybir.ActivationFunctionType.Tanh)
            warm2 = soft.tile([P, 1], F32, tag="warm2")
            nc.scalar.copy(warm2, maskb_sb)

            # ---- projections: qT[u,t], kT[u,s] (PE, contraction over h) ----
            psq = pproj.tile([P, UH, TQ], F32, tag="psq")
            psk = pproj.tile([P, UH, TK], F32, tag="psk")
            for uh in range(UH):
                for hc in range(HC):
                    nc.tensor.matmul(
                        psq[:, uh, :],
                        lhsT=w1_sb[:, hc, uh * P : (uh + 1) * P],
                        rhs=qt_sb[:, hc, :],
                        start=(hc == 0),
                        stop=(hc == HC - 1),
                    )
            for uh in range(UH):
                for hc in range(HC):
                    nc.tensor.matmul(
                        psk[:, uh, :],
                        lhsT=w2_sb[:, hc, uh * P : (uh + 1) * P],
                        rhs=vt_sb[:, hc, :],
                        start=(hc == 0),
                        stop=(hc == HC - 1),
                    )
            # PSUM -> SBUF copies on ACT (DVE is the cube bottleneck).
            qT_sb = consts.tile([P, UH, TQ], F32, tag="qT")
            kT_sb = consts.tile([P, UH, TK], F32, tag="kT")
            for uh in range(UH):
                nc.scalar.copy(qT_sb[:, uh, :], psq[:, uh, :])
                nc.scalar.copy(kT_sb[:, uh, :], psk[:, uh, :])

            # ---- main cube + per-half softmax/context tail ----
            # Separate PSUM tiles per t-half so the first half's exp read
            # doesn't serialize against the second half's matmul writes.
            scT = [
                psc.tile([P, TH], F32, tag="scT0", name="scT0"),
                psc.tile([P, TH], F32, tag="scT1", name="scT1"),
            ]
            expT_sb = soft.tile([P, TQ], F32, tag="expT")
            # Per-half tail tiles, all at partition base 0 (transpose matmul
            # outputs must start at PSUM partition 0); the output DMA remaps
            # rows to the right half of out_t.
            ctxp = [
                ptail.tile([TH, NH], F32, tag="ctx0", name="ctx0"),
                ptail.tile([TH, NH], F32, tag="ctx1", name="ctx1"),
            ]
            expp = [
                ptail.tile([TH, TK], F32, tag="exp0", name="exp0"),
                ptail.tile([TH, TK], F32, tag="exp1", name="exp1"),
            ]
            sums = [
                soft.tile([TH, 1], F32, tag="sums0", name="sums0"),
                soft.tile([TH, 1], F32, tag="sums1", name="sums1"),
            ]
            rinv = [
                soft.tile([TH, 1], F32, tag="rinv0", name="rinv0"),
                soft.tile([TH, 1], F32, tag="rinv1", name="rinv1"),
            ]
            outb = [
                soft.tile([TH, NH + TK], F32, tag="outb0", name="outb0"),
                soft.tile([TH, NH + TK], F32, tag="outb1", name="outb1"),
            ]

            def tail_dve(th):
                # DVE executes in order: these run late so they never
                # head-of-line-block the remaining chunks' adds.
                h0 = th * TH
                nc.vector.reduce_sum(sums[th], expp[th], axis=mybir.AxisListType.X)
                nc.vector.reciprocal(rinv[th], sums[th])
                nc.vector.tensor_scalar_mul(
                    outb[th][:, NH : NH + TK], expp[th], rinv[th]
                )
                nc.vector.tensor_scalar_mul(outb[th][:, 0:NH], ctxp[th], rinv[th])
                nc.sync.dma_start(out=out_t[h0 : h0 + TH, :], in_=outb[th])

            pending = None
            for ci in range(NCH):
                t0 = ci * TCH
                th = t0 // TH          # which t-half this chunk belongs to
                tl = t0 % TH           # position within the half
                tanh_t = []
                for uh in range(UH):
                    # bf16 S keeps all 16 tiles resident (no slot reuse ->
                    # no extra sync waits); tanh input rounding ~2e-3.
                    s_t = spool.tile([P, TCH, TK], BF16, tag="S")
                    kb = _bcast_free(kT_sb[:, uh, :], TCH, inner=False)
                    qb = _bcast_free(qT_sb[:, uh, t0 : t0 + TCH], TK, inner=True)
                    nc.vector.tensor_add(s_t, kb, qb)
                    t_t = tpool.tile([P, TCH, TK], BF16, tag="T")
                    nc.scalar.activation(t_t, s_t, mybir.ActivationFunctionType.Tanh)
                    tanh_t.append(t_t)
                if pending is not None and ci == pending + 2:
                    tail_dve(0)
                    pending = None
                for tloc in range(TCH):
                    for uh in range(UH):
                        nc.tensor.matmul(
                            scT[th][:, tl + tloc : tl + tloc + 1],
                            lhsT=tanh_t[uh][:, tloc, :],
                            rhs=scale_bf[:, uh : uh + 1],
                            start=(uh == 0),
                            stop=(uh == UH - 1),
                        )

                if tl + TCH == TH:
                    # This t-half's scores are complete: run its masked
                    # softmax + context (ACT/PE work) now so it overlaps the
                    # next half; the DVE part is deferred two chunks.
                    h0 = th * TH
                    esl = expT_sb[:, h0 : h0 + TH]
                    nc.scalar.activation(
                        esl, scT[th], mybir.ActivationFunctionType.Exp,
                        bias=maskb_sb,
                    )
                    nc.tensor.matmul(
                        ctxp[th], lhsT=esl, rhs=v_sb, start=True, stop=True
                    )
                    nc.tensor.transpose(expp[th], esl, ident)
                    if th == 0:
                        pending = ci
                    else:
                        tail_dve(1)

    nc.compile()
    return nc


_NC_CACHE = None


def _get_program():
    global _NC_CACHE
    if _NC_CACHE is None:
        _NC_CACHE = build_program()
    return _NC_CACHE


def make_in_maps(query, value, mask, W1, W2, scale):
    maskb = np.where(mask, 0.0, NEG_INF).astype(np.float32)
    w1 = np.asarray(W1, dtype=np.float32)
    w2 = np.asarray(W2, dtype=np.float32)
    sc2 = np.asarray(scale, dtype=np.float32).reshape(UH, P).T  # (128, 2)
    eye = np.eye(P, dtype=np.float32)
    in_maps = []
    for b in range(B):
        # (NH, PK) h-major -> (P, HC*PK) partition-major
        p1 = np.concatenate([query[b].T, w1], axis=1).reshape(HC, P, PK)
        p2 = np.concatenate([value[b].T, w2], axis=1).reshape(HC, P, PK)
        p1 = np.ascontiguousarray(p1.transpose(1, 0, 2).reshape(P, HC * PK))
        p2 = np.ascontiguousarray(p2.transpose(1, 0, 2).reshape(P, HC * PK))
        vaux = np.ascontiguousarray(
            np.concatenate(
                [np.asarray(value[b], dtype=np.float32), maskb[b][:, None], sc2, eye],
                axis=1,
            )
        )
        import ml_dtypes
        in_maps.append({"p1": p1.astype(ml_dtypes.bfloat16),
                        "p2": p2.astype(ml_dtypes.bfloat16),
                        "vaux": vaux})
    return in_maps


def kernel(query, value, mask, W1, W2, scale, **run_kwargs):
    query = np.asarray(query)
    value = np.asarray(value)
    mask = np.asarray(mask)
    nc = _get_program()
    in_maps = make_in_maps(query, value, mask, W1, W2, scale)
    res = run_bass_kernel_spmd(nc, in_maps, list(range(B)), **run_kwargs)
    context = np.stack([res.results[b]["out"][:, 0:NH] for b in range(B)])
    attn = np.stack([res.results[b]["out"][:, NH : NH + TK] for b in range(B)])
    kernel.last_results = res
    return context, attn


# revision 28
# speedup vs baseline: 1.2693x; 1.0374x over previous
"""Bahdanau additive attention on 8 Trainium2 NeuronCores.

Reference computation (per batch b):
    q = query[b] @ W1                      # (TQ, U)
    k = value[b] @ W2                      # (TK, U)
    scores[t,s] = sum_u scale[u] * tanh(q[t,u] + k[s,u])
    attn = softmax(scores + mask_bias, axis=s)
    context = attn @ value[b]              # (TQ, NH)

Sharding: pure data-parallel over batch (B=8 == n_cores). Each core gets
its own batch slice plus replicated W1/W2/scale; no collectives.

Per-core dataflow (partition dim = u for the cube stages):
    PE:   qT[u,t] = W1^T q^T,  kT[u,s] = W2^T v^T   (contraction over h)
    DVE:  S[u, (t,s)] = qT[u,t] (bcast over s) + kT[u,s] (bcast over t)
    ACT:  T = tanh(S)  (bf16)
    PE:   scoresT[s, t] (PSUM) += tanh_slice(u,s)^T @ scale_half(u,1),
          accumulated over the two u-halves, one matmul per (t, half)
    per t-half (so the first half's tail hides under the second half's
    cube work):
      ACT:  expT = exp(scoresT + maskb[s])   (mask folded into the bias)
      PE:   ctx_unnorm(t,h) = expT^T @ v ; sums(t,1) = expT^T @ ones
            exp(t,s) = transpose(expT)
      DVE:  rinv = 1/sums ; attn = exp * rinv ; ctx = ctx_unnorm * rinv

Host packs inputs partition-major so each DMA descriptor covers a multi-KB
contiguous run (the DMA engines are descriptor-rate-bound otherwise).
"""

import numpy as np

import concourse.bass as bass
import concourse.mybir as mybir
import concourse.tile as tile
from concourse import bacc
from concourse.bass_utils import run_bass_kernel_spmd

F32 = mybir.dt.float32
BF16 = mybir.dt.bfloat16

P = 128          # partitions
B = 8            # batch == n_cores
TQ = 128         # query positions
TK = 128         # key positions
NH = 512         # model dim
U = 256          # attention units
HC = NH // P     # h chunks (4)
UH = U // P      # u halves (2)
# Variable t-chunk sizes: small chunks at the start (fast pipeline fill)
# and at the end (short serial tail after the last add).
CHUNKS = [8, 8, 16, 16, 16, 16, 16, 16, 8, 8]
NCH = len(CHUNKS)
TH = TQ // 2     # tail processed per t-half (64)
NEG_INF = -1e9

PK = TQ + U              # packed projection width (384)
# vaux packed columns: [ value(512) | maskb(1) | scale(2) | identity(128) ]
VA_V, VA_MB, VA_SC, VA_ID = 0, NH, NH + 1, NH + 3
VA_W = NH + 3 + P        # 643


def _bcast_free(sub, n, inner):
    """Insert a broadcast (step-0) free dim into an AP.

    inner=True appends [0, n] as the innermost free dim; inner=False puts
    it as the outermost free dim (right after the partition dim).
    """
    ap = [list(d) for d in sub.ap]
    if inner:
        new = ap + [[0, n]]
    else:
        new = [ap[0], [0, n]] + ap[1:]
    return bass.AP(tensor=sub.tensor, offset=sub.offset, ap=new)


def build_program():
    nc = bacc.Bacc("TRN2", target_bir_lowering=False)

    # p1 = [query_b^T | W1], p2 = [value_b^T | W2], both stored
    # partition-major on the host: row p holds the four h-chunks
    # (h = c*128 + p) back to back -> 1.5KB-6KB contiguous DMA runs.
    p1 = nc.dram_tensor("p1", [P, HC * PK], BF16, kind="ExternalInput").ap()
    p2 = nc.dram_tensor("p2", [P, HC * PK], BF16, kind="ExternalInput").ap()
    vaux = nc.dram_tensor("vaux", [TK, VA_W], F32, kind="ExternalInput").ap()
    out_t = nc.dram_tensor("out", [TQ, NH + TK], F32, kind="ExternalOutput").ap()

    with tile.TileContext(nc) as tc:
        with (
            tc.tile_pool(name="consts", bufs=1) as consts,
            tc.tile_pool(name="spool", bufs=NCH * UH) as spool,
            tc.tile_pool(name="tpool", bufs=NCH * UH) as tpool,
            tc.tile_pool(name="soft", bufs=1) as soft,
            tc.tile_pool(name="pproj", bufs=1, space="PSUM") as pproj,
            tc.tile_pool(name="psc", bufs=1, space="PSUM") as psc,
            tc.tile_pool(name="ptail", bufs=1, space="PSUM") as ptail,
        ):
            # ---- input loads ----
            p1_sb = consts.tile([P, HC, PK], BF16, tag="p1")
            p2_sb = consts.tile([P, HC, PK], BF16, tag="p2")
            va_sb = consts.tile([P, VA_W], F32, tag="vaux")

            # Two DMAs per projection tensor (3KB descriptor runs) so the
            # first half's projections can start while the rest streams in.
            half = HC // 2 * PK
            nc.sync.dma_start(out=p1_sb[:, 0 : HC // 2, :], in_=p1[:, 0:half])
            nc.sync.dma_start(out=p2_sb[:, 0 : HC // 2, :], in_=p2[:, 0:half])
            nc.sync.dma_start(out=p1_sb[:, HC // 2 : HC, :], in_=p1[:, half:])
            nc.sync.dma_start(out=p2_sb[:, HC // 2 : HC, :], in_=p2[:, half:])
            nc.sync.dma_start(out=va_sb, in_=vaux)

            qt_sb = p1_sb[:, :, 0:TQ]
            w1_sb = p1_sb[:, :, TQ:PK]
            vt_sb = p2_sb[:, :, 0:TK]
            w2_sb = p2_sb[:, :, TK:PK]
            v_sb = va_sb[:, VA_V : VA_V + NH]
            maskb_sb = va_sb[:, VA_MB : VA_MB + 1]
            scale_f32 = va_sb[:, VA_SC : VA_SC + UH]
            ident = va_sb[:, VA_ID : VA_ID + P]

            scale_bf = consts.tile([P, UH], BF16, tag="scalebf")
            nc.vector.tensor_copy(scale_bf, scale_f32)
            # Warm the ACT table set (tanh/exp share "exp_and_others") while
            # the input DMAs are in flight; also touch maskb on ACT early.
            warm = soft.tile([P, 1], F32, tag="warm")
            nc.vector.memset(warm, 0.0)
            nc.scalar.activation(warm, warm, mybir.ActivationFunctionType.Tanh)
            warm2 = soft.tile([P, 1], F32, tag="warm2")
            nc.scalar.copy(warm2, maskb_sb)

            # ---- projections: qT[u,t], kT[u,s] (PE, contraction over h) ----
            psq = pproj.tile([P, UH, TQ], F32, tag="psq")
            psk = pproj.tile([P, UH, TK], F32, tag="psk")
            for uh in range(UH):
                for hc in range(HC):
                    nc.tensor.matmul(
                        psq[:, uh, :],
                        lhsT=w1_sb[:, hc, uh * P : (uh + 1) * P],
                        rhs=qt_sb[:, hc, :],
                        start=(hc == 0),
                        stop=(hc == HC - 1),
                    )
            for uh in range(UH):
                for hc in range(HC):
                    nc.tensor.matmul(
                        psk[:, uh, :],
                        lhsT=w2_sb[:, hc, uh * P : (uh + 1) * P],
                        rhs=vt_sb[:, hc, :],
                        start=(hc == 0),
                        stop=(hc == HC - 1),
                    )
            # PSUM -> SBUF copies on ACT (DVE is the cube bottleneck).
            qT_sb = consts.tile([P, UH, TQ], F32, tag="qT")
            kT_sb = consts.tile([P, UH, TK], F32, tag="kT")
            for uh in range(UH):
                nc.scalar.copy(qT_sb[:, uh, :], psq[:, uh, :])
                nc.vector.tensor_copy(kT_sb[:, uh, :], psk[:, uh, :])

            # ---- main cube + per-half softmax/context tail ----
            # Separate PSUM tiles per t-half so the first half's exp read
            # doesn't serialize against the second half's matmul writes.
            scT = [
                psc.tile([P, TH], F32, tag="scT0", name="scT0"),
                psc.tile([P, TH], F32, tag="scT1", name="scT1"),
            ]
            expT_sb = soft.tile([P, TQ], F32, tag="expT")
            # Per-half tail tiles, all at partition base 0 (transpose matmul
            # outputs must start at PSUM partition 0); the output DMA remaps
            # rows to the right half of out_t.
            ctxp = [
                ptail.tile([TH, NH], F32, tag="ctx0", name="ctx0"),
                ptail.tile([TH, NH], F32, tag="ctx1", name="ctx1"),
            ]
            expp = [
                ptail.tile([TH, TK], F32, tag="exp0", name="exp0"),
                ptail.tile([TH, TK], F32, tag="exp1", name="exp1"),
            ]
            sums = [
                soft.tile([TH, 1], F32, tag="sums0", name="sums0"),
                soft.tile([TH, 1], F32, tag="sums1", name="sums1"),
            ]
            rinv = [
                soft.tile([TH, 1], F32, tag="rinv0", name="rinv0"),
                soft.tile([TH, 1], F32, tag="rinv1", name="rinv1"),
            ]
            outb = [
                soft.tile([TH, NH + TK], F32, tag="outb0", name="outb0"),
                soft.tile([TH, NH + TK], F32, tag="outb1", name="outb1"),
            ]

            def tail_dve(th):
                # DVE executes in order: these run late so they never
                # head-of-line-block the remaining chunks' adds.
                h0 = th * TH
                nc.vector.reduce_sum(sums[th], expp[th], axis=mybir.AxisListType.X)
                nc.vector.reciprocal(rinv[th], sums[th])
                nc.vector.tensor_scalar_mul(
                    outb[th][:, NH : NH + TK], expp[th], rinv[th]
                )
                nc.vector.tensor_scalar_mul(outb[th][:, 0:NH], ctxp[th], rinv[th])
                nc.sync.dma_start(out=out_t[h0 : h0 + TH, :], in_=outb[th])

            pending = None
            t0 = 0
            for ci, tch in enumerate(CHUNKS):
                th = t0 // TH          # which t-half this chunk belongs to
                tl = t0 % TH           # position within the half
                tanh_t = []
                for uh in range(UH):
                    # bf16 S tiles all stay resident (no slot reuse ->
                    # no extra sync waits); tanh input rounding ~2e-3.
                    s_t = spool.tile([P, tch, TK], BF16, tag="S", name=f"s_{ci}_{uh}")
                    kb = _bcast_free(kT_sb[:, uh, :], tch, inner=False)
                    qb = _bcast_free(qT_sb[:, uh, t0 : t0 + tch], TK, inner=True)
                    nc.vector.tensor_add(s_t, kb, qb)
                    t_t = tpool.tile([P, tch, TK], BF16, tag="T", name=f"t_{ci}_{uh}")
                    nc.scalar.activation(t_t, s_t, mybir.ActivationFunctionType.Tanh)
                    tanh_t.append(t_t)
                if pending is not None and ci == pending + 2:
                    tail_dve(0)
                    pending = None
                for tloc in range(tch):
                    for uh in range(UH):
                        nc.tensor.matmul(
                            scT[th][:, tl + tloc : tl + tloc + 1],
                            lhsT=tanh_t[uh][:, tloc, :],
                            rhs=scale_bf[:, uh : uh + 1],
                            start=(uh == 0),
                            stop=(uh == UH - 1),
                        )

                t0 += tch
                if tl + tch == TH:
                    # This t-half's scores are complete: run its masked
                    # softmax + context (ACT/PE work) now so it overlaps the
                    # next half; the DVE part is deferred two chunks.
                    h0 = th * TH
                    esl = expT_sb[:, h0 : h0 + TH]
                    nc.scalar.activation(
                        esl, scT[th], mybir.ActivationFunctionType.Exp,
                        bias=maskb_sb,
                    )
                    nc.tensor.matmul(
                        ctxp[th], lhsT=esl, rhs=v_sb, start=True, stop=True
                    )
                    nc.tensor.transpose(expp[th], esl, ident)
                    if th == 0:
                        pending = ci
                    else:
                        tail_dve(1)

    nc.compile()
    return nc


_NC_CACHE = None


def _get_program():
    global _NC_CACHE
    if _NC_CACHE is None:
        _NC_CACHE = build_program()
    return _NC_CACHE


def make_in_maps(query, value, mask, W1, W2, scale):
    maskb = np.where(mask, 0.0, NEG_INF).astype(np.float32)
    w1 = np.asarray(W1, dtype=np.float32)
    w2 = np.asarray(W2, dtype=np.float32)
    sc2 = np.asarray(scale, dtype=np.float32).reshape(UH, P).T  # (128, 2)
    eye = np.eye(P, dtype=np.float32)
    in_maps = []
    for b in range(B):
        # (NH, PK) h-major -> (P, HC*PK) partition-major
        p1 = np.concatenate([query[b].T, w1], axis=1).reshape(HC, P, PK)
        p2 = np.concatenate([value[b].T, w2], axis=1).reshape(HC, P, PK)
        p1 = np.ascontiguousarray(p1.transpose(1, 0, 2).reshape(P, HC * PK))
        p2 = np.ascontiguousarray(p2.transpose(1, 0, 2).reshape(P, HC * PK))
        vaux = np.ascontiguousarray(
            np.concatenate(
                [np.asarray(value[b], dtype=np.float32), maskb[b][:, None], sc2, eye],
                axis=1,
            )
        )
        import ml_dtypes
        in_maps.append({"p1": p1.astype(ml_dtypes.bfloat16),
                        "p2": p2.astype(ml_dtypes.bfloat16),
                        "vaux": vaux})
    return in_maps


def kernel(query, value, mask, W1, W2, scale, **run_kwargs):
    query = np.asarray(query)
    value = np.asarray(value)
    mask = np.asarray(mask)
    nc = _get_program()
    in_maps = make_in_maps(query, value, mask, W1, W2, scale)
    res = run_bass_kernel_spmd(nc, in_maps, list(range(B)), **run_kwargs)
    context = np.stack([res.results[b]["out"][:, 0:NH] for b in range(B)])
    attn = np.stack([res.results[b]["out"][:, NH : NH + TK] for b in range(B)])
    kernel.last_results = res
    return context, attn
